# revision 22
# baseline (speedup 1.0000x reference)
"""EnhancedAttention Trainium2 kernel (nn_EnhancedAttention_70068096467384).

Sharding: 8 cores = 2 batches x 4 query-slices (256 queries each).
Each core computes the full K/V projections for its batch (duplicated
within the 4-core batch group; platform collectives have ~80us fixed
overhead, more than the whole kernel target), attention for its query
slice over all 16 heads, the output projection, residual and LayerNorm,
and returns its [256, 1024] slice of the output. The host concatenates
slices -- pure data movement, no arithmetic.

Layout: activations feature-major ("transposed" [feature, token]) so
every matmul contracts over the partition dim:
  Q^T[d,q]   = Wq.T @ qslice^T         (lhsT=Wq block,   rhs=query^T slice)
  K^T[d,k]   = Wk.T @ key^T
  V[k,d]     = value^T.T @ Wv          (lhsT=value^T,    rhs=Wv block)
  s^T[k,q]   = (K^T).T @ Q^T           (per head, contraction d=64)
  ctx^T[d,q] = [V|1].T @ exp(s')       (ones column yields softmax sums)
  out[s,h]   = (ctx^T).T @ Wo          (token-major again for LayerNorm)

Gate math (per-head msb scalar a, per-batch scalar spec):
  scores' = spec * s * (1 + SP*sigmoid(a*s)),  s = Q K^T / sqrt(HD)
  with sigmoid(z) = (1+tanh(z/2))/2:
  scores' = A*s + B*s*v,  v = tanh((a/2)*s),  A = spec*(1+SP/2), B = spec*SP/2
  exp(scores') = Exp(A * g),  g = s * (1 + (B/A)*v),  B/A const = (SP/2)/(1+SP/2)
tanh and exp share one ACT table set (exp_and_others) -> no table
ping-pong. Softmax skips the row-max subtraction (scores are bounded,
|scores'| < ~3), so unnormalized exps are valid and the ones-column sums
normalize ctx. 1/sum is applied to ctx^T via a PE broadcast of the
reciprocal row. rstd for LayerNorm = Exp(-0.5*Ln(var+eps)) (ln/exp share
a table set; avoids the loose-ULP sqrt table).
"""

import numpy as np

B, S, H, NH = 2, 1024, 1024, 16
HD = H // NH            # 64
H2 = H // 2             # 512 (spec MLP hidden)
SP = 0.05
EPS = 1e-5
P = 128
NCH = H // P            # 8 feature chunks
NKB = S // P            # 8 key blocks
QSHARD = 4
QSL = S // QSHARD       # 256
BA = (SP / 2.0) / (1.0 + SP / 2.0)
AF = 1.0 + SP / 2.0
MM_DT = "float32r"      # fast fp32 matmul mode; "float32" = exact but 4x slower

_CACHE = {}


def _build(mm_dt=MM_DT):
    import concourse.bacc as bacc
    import concourse.mybir as mybir
    import concourse.tile as tile

    f32 = mybir.dt.float32
    bf16 = mybir.dt.bfloat16
    mmdt = getattr(mybir.dt, mm_dt)
    A = mybir.AluOpType
    AT = mybir.ActivationFunctionType

    def r(ap):
        return ap.bitcast(mmdt)

    nc = bacc.Bacc(None, target_bir_lowering=False, debug=False)

    def din(name, shape):
        return nc.dram_tensor(name, shape, f32, kind="ExternalInput").ap()

    def dinr(name, shape):
        return nc.dram_tensor(name, shape, mmdt, kind="ExternalInput").ap()

    def dinb(name, shape):
        return nc.dram_tensor(name, shape, bf16, kind="ExternalInput").ap()

    qT = dinb("qT", [H, S])          # query^T full (spec-MLP mean)
    qsT = dinr("qsT", [H, QSL])      # query^T slice (Q projection)
    kT = dinr("kT", [H, S])
    vT = dinb("vT", [H, S])
    qres = din("qres", [QSL, H])    # query slice token-major (residual)
    Wq, Wk, Wo = (dinr(n, [H, H]) for n in ("Wq", "Wk", "Wo"))
    Wv = dinb("Wv", [H, H])
    Ws1 = dinb("Ws1", [H, H2])
    Ws2 = dinb("Ws2", [H2, H])
    bqc = din("bqc", [P, NCH])      # bq.reshape(8,128).T
    bkc = din("bkc", [P, NCH])
    bs1r = din("bs1r", [1, H2])
    bs2r = din("bs2r", [1, H])
    bvb = din("bvb", [P, H])        # broadcasts along partitions
    bob = din("bob", [P, H])
    lgb = din("lgb", [P, H])
    lbb = din("lbb", [P, H])
    msbr = din("msbr", [P, NH * HD * HD // P])   # msb flat as [128, 512]
    gsel = din("gsel", [P, NH])     # gsel[p,h] = (p//8 == h)
    eye = din("eye", [HD, HD])
    out = nc.dram_tensor("out", [QSL, H], f32, kind="ExternalOutput").ap()

    qTc = qT.rearrange("(c p) s -> c p s", p=P)
    qsTc = qsT.rearrange("(c p) s -> c p s", p=P)
    kTc = kT.rearrange("(c p) s -> c p s", p=P)
    vTc = vT.rearrange("(c p) s -> c p s", p=P)
    Wqc = Wq.rearrange("(c p) n -> c p n", p=P)
    Wkc = Wk.rearrange("(c p) n -> c p n", p=P)
    Wvc = Wv.rearrange("(c p) n -> c p n", p=P)
    Woc = Wo.rearrange("(c p) n -> c p n", p=P)
    Ws1c = Ws1.rearrange("(c p) n -> c p n", p=P)
    Ws2c = Ws2.rearrange("(c p) n -> c p n", p=P)
    qresc = qres.rearrange("(c p) n -> c p n", p=P)
    outc = out.rearrange("(c p) n -> c p n", p=P)

    from contextlib import ExitStack

    with tile.TileContext(nc) as tc:
        with ExitStack() as ctx:
            ec = ctx.enter_context
            consts = ec(tc.tile_pool(name="consts", bufs=1))
            actin = ec(tc.tile_pool(name="actin", bufs=8))
            qsin = ec(tc.tile_pool(name="qsin", bufs=NCH))
            wstr = ec(tc.tile_pool(name="wstr", bufs=8))
            ktp = ec(tc.tile_pool(name="ktp", bufs=NCH))
            vaugp = ec(tc.tile_pool(name="vaugp", bufs=NKB))
            qtp = ec(tc.tile_pool(name="qtp", bufs=NCH))
            ctxp = ec(tc.tile_pool(name="ctxp", bufs=NCH))
            gate3 = ec(tc.tile_pool(name="gate3", bufs=3))
            gate2 = ec(tc.tile_pool(name="gate2", bufs=1))
            pexp = ec(tc.tile_pool(name="pexp", bufs=3))
            smalls = ec(tc.tile_pool(name="smalls", bufs=1))
            epil = ec(tc.tile_pool(name="epil", bufs=2))
            wmlp = ec(tc.tile_pool(name="wmlp", bufs=2))
            ps_sc = ec(tc.tile_pool(name="ps_sc", bufs=3, space="PSUM"))
            ps_pv = ec(tc.tile_pool(name="ps_pv", bufs=2, space="PSUM"))
            ps_big = ec(tc.tile_pool(name="ps_big", bufs=2, space="PSUM"))
            ps_sm = ec(tc.tile_pool(name="ps_sm", bufs=1, space="PSUM"))
            ps_bc = ps_sm
            # ---------------- constants ----------------
            ones64 = consts.tile([P, HD], f32)
            nc.vector.memset(ones64, 1.0)
            onesrow = consts.tile([1, P], f32)
            nc.vector.memset(onesrow, 1.0)
            one1 = consts.tile([1, 1], f32)
            nc.vector.memset(one1, 1.0)
            eps_vec = consts.tile([P, 1], f32)
            nc.vector.memset(eps_vec, EPS)
            bq_sb = consts.tile([P, NCH], f32)
            nc.gpsimd.dma_start(out=bq_sb, in_=bqc)
            bk_sb = consts.tile([P, NCH], f32)
            nc.gpsimd.dma_start(out=bk_sb, in_=bkc)
            bs1_sb = consts.tile([1, H2], f32)
            nc.gpsimd.dma_start(out=bs1_sb, in_=bs1r)
            bs2_sb = consts.tile([1, H], f32)
            nc.gpsimd.dma_start(out=bs2_sb, in_=bs2r)
            gsel_sb = consts.tile([P, NH], f32)
            nc.gpsimd.dma_start(out=gsel_sb, in_=gsel)
            eye_sb = consts.tile([HD, HD], f32)
            nc.gpsimd.dma_start(out=eye_sb, in_=eye)
            onescol = consts.tile([P, NH, 1], bf16)
            nc.vector.memset(onescol, 1.0)

            # -------- msb head scalars: ah[:,h] = mean(msb[h]) / 2 --------
            msb_sb = smalls.tile([P, NH * HD * HD // P], f32, tag="sm")
            nc.gpsimd.dma_start(out=msb_sb, in_=msbr)
            mpart = smalls.tile([P, 1], f32, tag="sm2")
            nc.vector.tensor_reduce(out=mpart, in_=msb_sb, op=A.add,
                                    axis=mybir.AxisListType.X)
            # scale by 1/(HD*HD) * 1/2 now (per-partition partial sums)
            nc.vector.tensor_scalar_mul(mpart, mpart, 0.5 / (HD * HD))
            ps_mh = ps_sm.tile([P, 512], f32, tag="ps")
            nc.tensor.matmul(ps_mh[0:NH, 0:1], gsel_sb, mpart,
                             start=True, stop=True)
            mh16 = smalls.tile([16, 1], f32, tag="sm3")
            nc.vector.tensor_copy(out=mh16, in_=ps_mh[0:NH, 0:1])
            ps_mr = ps_sm.tile([P, 512], f32, tag="ps")
            nc.tensor.matmul(ps_mr[0:1, 0:NH], mh16, eye_sb[0:NH, 0:NH],
                             start=True, stop=True)
            mrow = smalls.tile([1, NH], f32, tag="sm4")
            nc.vector.tensor_copy(out=mrow, in_=ps_mr[0:1, 0:NH])
            ps_ah = ps_sm.tile([P, 512], f32, tag="ps")
            nc.tensor.matmul(ps_ah[:, 0:NH], onesrow, mrow, start=True, stop=True)
            ah_sb = consts.tile([P, NH], f32)
            nc.vector.tensor_copy(out=ah_sb, in_=ps_ah[:, 0:NH])

            # -------- query^T slice (Q proj input) and full (spec mean) --------
            qs_in = []
            for c in range(NCH):
                t = qsin.tile([P, QSL], mmdt, tag="qs")
                nc.sync.dma_start(out=t, in_=qsTc[c])
                qs_in.append(t)
            sin_col = smalls.tile([P, NCH], bf16, tag="sin")
            with nc.allow_low_precision(
                    reason="spec-MLP input mean; feeds a sigmoid-mean scalar"):
                for c in range(NCH):
                    t = actin.tile([P, S], bf16, tag="act")
                    nc.sync.dma_start(out=t, in_=qTc[c])
                    nc.vector.tensor_reduce(out=sin_col[:, c:c + 1], in_=t,
                                            op=A.add, axis=mybir.AxisListType.X)

            # -------- Q^T projection (+bias, x 1/sqrt(HD)) --------
            qt = [qtp.tile([P, QSL], mmdt, tag="qt", name=f"qt{i}") for i in range(NCH)]
            for dh in range(2):
                wblk = []
                for c in range(NCH):
                    w = wstr.tile([P, 512], mmdt, tag="w")
                    nc.gpsimd.dma_start(out=w, in_=Wqc[c][:, dh * 512:(dh + 1) * 512])
                    wblk.append(w)
                for dbl in range(4):
                    db = dh * 4 + dbl
                    ps_q = ps_big.tile([P, 512], f32, tag="pb")
                    for c in range(NCH):
                        nc.tensor.matmul(
                            ps_q[:, 0:QSL],
                            r(wblk[c][:, dbl * P:(dbl + 1) * P]),
                            r(qs_in[c]),
                            start=(c == 0), stop=(c == NCH - 1))
                    nc.scalar.activation(
                        out=qt[db], in_=ps_q[:, 0:QSL], func=AT.Identity,
                        bias=bq_sb[:, db:db + 1], scale=1.0 / np.sqrt(HD))

            # -------- spec MLP --------
            ps_m1 = ps_big.tile([P, 512], f32, tag="pb")
            for c in range(NCH):
                w = wmlp.tile([P, 512], bf16, tag="wm")
                nc.gpsimd.dma_start(out=w, in_=Ws1c[c])
                nc.tensor.matmul(ps_m1[0:1, :], sin_col[:, c:c + 1], w,
                                 start=(c == 0), stop=(c == NCH - 1))
            h1row = smalls.tile([1, H2], f32, tag="h1r")
            nc.vector.scalar_tensor_tensor(
                out=h1row, in0=ps_m1[0:1, :], scalar=1.0 / S, in1=bs1_sb,
                op0=A.mult, op1=A.add)
            h1c = smalls.tile([P, 4], bf16, tag="h1c")
            for c in range(4):
                ps_tr = ps_sm.tile([P, 512], f32, tag="ps")
                nc.tensor.matmul(ps_tr[:, 0:1],
                                 h1row[0:1, c * P:(c + 1) * P], one1,
                                 start=True, stop=True)
                nc.vector.tensor_copy(out=h1c[:, c:c + 1], in_=ps_tr[:, 0:1])
            nc.vector.tensor_scalar_max(h1c, h1c, 0.0)
            zrow = smalls.tile([1, H], f32, tag="zr")
            for half in range(2):
                ps_m2 = ps_big.tile([P, 512], f32, tag="pb")
                for c in range(4):
                    w = wmlp.tile([P, 512], bf16, tag="wm")
                    nc.gpsimd.dma_start(out=w, in_=Ws2c[c][:, half * 512:(half + 1) * 512])
                    nc.tensor.matmul(ps_m2[0:1, :], h1c[:, c:c + 1], w,
                                     start=(c == 0), stop=(c == 3))
                nc.vector.tensor_add(
                    out=zrow[0:1, half * 512:(half + 1) * 512],
                    in0=ps_m2[0:1, :],
                    in1=bs2_sb[0:1, half * 512:(half + 1) * 512])
            zsig = smalls.tile([1, H], f32, tag="sm")
            nc.scalar.activation(out=zsig, in_=zrow, func=AT.Sigmoid)
            zsum = smalls.tile([1, 1], f32, tag="zsum")
            nc.vector.tensor_reduce(out=zsum, in_=zsig, op=A.add,
                                    axis=mybir.AxisListType.X)
            ps_sp = ps_sm.tile([P, 512], f32, tag="ps")
            nc.tensor.matmul(ps_sp[:, 0:1], onesrow, zsum, start=True, stop=True)
            a_vec = consts.tile([P, 1], f32)
            nc.vector.tensor_scalar_mul(a_vec, ps_sp[:, 0:1], AF / H)

            # -------- V projection -> V_aug = per head [V|1] / [1|V] --------
            bvb_sb = consts.tile([P, H], f32)
            nc.gpsimd.dma_start(out=bvb_sb, in_=bvb)
            vt_in = []
            for c in range(NCH):
                t = actin.tile([P, S], bf16, tag="act")
                nc.sync.dma_start(out=t, in_=vTc[c])
                vt_in.append(t)
            vaug = [vaugp.tile([P, NH, HD + 1], bf16, tag="va", name=f"va{i}") for i in range(NKB)]
            for kb in range(NKB):
                nc.vector.tensor_copy(out=vaug[kb][:, :, HD:HD + 1],
                                      in_=onescol)                 # [V_h | 1]
            for dh in range(2):
                wblk = []
                for c in range(NCH):
                    w = wstr.tile([P, 512], bf16, tag="w")
                    nc.gpsimd.dma_start(out=w, in_=Wvc[c][:, dh * 512:(dh + 1) * 512])
                    wblk.append(w)
                for kb in range(NKB):
                    ps_v = ps_big.tile([P, 512], f32, tag="pb")
                    for c in range(NCH):
                        nc.tensor.matmul(
                            ps_v,
                            vt_in[c][:, kb * P:(kb + 1) * P],
                            wblk[c],
                            start=(c == 0), stop=(c == NCH - 1))
                    psv = ps_v.rearrange("p (g w) -> p g w", w=HD)
                    bvv = bvb_sb[:, dh * 512:(dh + 1) * 512].rearrange(
                        "p (g w) -> p g w", w=HD)
                    nc.vector.tensor_add(
                        out=vaug[kb][:, dh * 8:dh * 8 + 8, 0:HD],
                        in0=psv, in1=bvv)

            # -------- K^T projection (+bias) --------
            kt_in = []
            for c in range(NCH):
                t = actin.tile([P, S], mmdt, tag="act")
                nc.sync.dma_start(out=t, in_=kTc[c])
                kt_in.append(t)
            kt = [ktp.tile([P, S], mmdt, tag="kt", name=f"kt{i}") for i in range(NCH)]
            for dh in range(2):
                wblk = []
                for c in range(NCH):
                    w = wstr.tile([P, 512], mmdt, tag="w")
                    nc.gpsimd.dma_start(out=w, in_=Wkc[c][:, dh * 512:(dh + 1) * 512])
                    wblk.append(w)
                for dbl in range(4):
                    db = dh * 4 + dbl
                    for kh in range(2):
                        ps_k = ps_big.tile([P, 512], f32, tag="pb")
                        for c in range(NCH):
                            nc.tensor.matmul(
                                ps_k,
                                r(wblk[c][:, dbl * P:(dbl + 1) * P]),
                                r(kt_in[c][:, kh * 512:(kh + 1) * 512]),
                                start=(c == 0), stop=(c == NCH - 1))
                        nc.scalar.activation(
                            out=kt[db][:, kh * 512:(kh + 1) * 512], in_=ps_k,
                            func=AT.Identity, bias=bk_sb[:, db:db + 1],
                            scale=1.0)

            # -------- attention heads (+ interleaved out-proj half 0) --------
            wo0 = []
            for c in range(NCH):
                w = wstr.tile([P, 512], mmdt, tag="w")
                nc.gpsimd.dma_start(out=w, in_=Woc[c][:, 0:512])
                wo0.append(w)
            pso0 = [ps_big.tile([P, 512], f32, tag="pb", name=f"pso0_{i}")
                    for i in range(2)]
            ctxt = [ctxp.tile([P, QSL], mmdt, tag="ctx", name=f"ctx{i}") for i in range(NCH)]
            for h in range(NH):
                ch, off = h // 2, (h % 2) * HD
                even = (h % 2 == 0)
                pv_ps = ps_pv.tile([P, QSL], f32, tag="pv")
                for kp in range(NKB // 2):
                    s_ps = ps_sc.tile([P, 2 * QSL], f32, tag="sc")
                    for j in range(2):
                        kb = 2 * kp + j
                        nc.tensor.matmul(
                            s_ps[:, j * QSL:(j + 1) * QSL],
                            r(kt[ch][off:off + HD, kb * P:(kb + 1) * P]),
                            r(qt[ch][off:off + HD, :]), start=True, stop=True)
                    v_sb = gate3.tile([P, 2 * QSL], bf16, tag="v")
                    nc.scalar.activation(out=v_sb, in_=s_ps, func=AT.Tanh,
                                         scale=ah_sb[:, h:h + 1])
                    w1_sb = gate3.tile([P, 2 * QSL], bf16, tag="w1")
                    nc.vector.tensor_scalar(
                        out=w1_sb, in0=v_sb, scalar1=BA, scalar2=1.0,
                        op0=A.mult, op1=A.add)
                    g_sb = gate3.tile([P, 2 * QSL], f32, tag="g")
                    nc.vector.tensor_mul(out=g_sb, in0=s_ps, in1=w1_sb)
                    p_sb = pexp.tile([P, 2 * QSL], bf16, tag="p")
                    nc.scalar.activation(out=p_sb, in_=g_sb, func=AT.Exp,
                                         scale=a_vec)
                    for j in range(2):
                        kb = 2 * kp + j
                        lh = vaug[kb].rearrange("p h w -> p (h w)")
                        nc.tensor.matmul(
                            pv_ps[0:HD + 1, :],
                            lh[:, h * (HD + 1):(h + 1) * (HD + 1)],
                            p_sb[:, j * QSL:(j + 1) * QSL],
                            start=(kb == 0), stop=(kb == NKB - 1))
                # normalize ctx rows by softmax sums (row HD of pv_ps)
                inv_sb = gate2.tile([P, QSL], f32, tag="inv")
                nc.vector.reciprocal(out=inv_sb[HD:HD + 1, :],
                                     in_=pv_ps[HD:HD + 1, :])
                bc_ps = ps_bc.tile([P, 2 * QSL], f32, tag="ps")
                nc.tensor.matmul(
                    bc_ps[0:HD, 0:QSL], ones64[HD:HD + 1, 0:HD],
                    inv_sb[HD:HD + 1, :], start=True, stop=True)
                bc_sb = gate2.tile([P, QSL], f32, tag="bcs")
                nc.vector.tensor_copy(out=bc_sb[0:HD, :], in_=bc_ps[0:HD, 0:QSL])
                if even:
                    nc.vector.tensor_mul(
                        out=ctxt[ch][0:HD, :],
                        in0=pv_ps[0:HD, :], in1=bc_sb[0:HD, :])
                else:
                    # scale into a temp, then PE-shift to partitions 64..127
                    cso = gate2.tile([P, QSL], f32, tag="cso")
                    nc.vector.tensor_mul(
                        out=cso[0:HD, :], in0=pv_ps[0:HD, :],
                        in1=bc_sb[0:HD, :])
                    sh_ps = ps_bc.tile([P, 2 * QSL], f32, tag="ps")
                    nc.tensor.matmul(
                        sh_ps[HD:P, 0:QSL], eye_sb, cso[0:HD, :],
                        start=True, stop=True)
                    nc.vector.tensor_copy(out=ctxt[ch][HD:P, :],
                                          in_=sh_ps[HD:P, 0:QSL])
                if not even:
                    for sb in range(2):
                        nc.tensor.matmul(
                            pso0[sb], r(ctxt[ch][:, sb * P:(sb + 1) * P]),
                            r(wo0[ch]),
                            start=(ch == 0), stop=(ch == NCH - 1))

            # -------- output projection + residual + LayerNorm --------
            bob_sb = consts.tile([P, H], f32)
            nc.gpsimd.dma_start(out=bob_sb, in_=bob)
            lgb_sb = consts.tile([P, H], f32)
            nc.gpsimd.dma_start(out=lgb_sb, in_=lgb)
            lbb_sb = consts.tile([P, H], f32)
            nc.gpsimd.dma_start(out=lbb_sb, in_=lbb)
            osbs = []
            for sb in range(2):
                osbs.append(epil.tile([P, H], f32, tag="osb", name=f"osb{sb}"))
            for sb in range(2):
                qr = epil.tile([P, 512], f32, tag="qr")
                nc.sync.dma_start(out=qr, in_=qresc[sb][:, 0:512])
                nc.vector.tensor_add(out=osbs[sb][:, 0:512], in0=pso0[sb],
                                     in1=qr)
                nc.vector.tensor_add(out=osbs[sb][:, 0:512],
                                     in0=osbs[sb][:, 0:512],
                                     in1=bob_sb[:, 0:512])
            wo1 = []
            for c in range(NCH):
                w = wstr.tile([P, 512], mmdt, tag="w")
                nc.gpsimd.dma_start(out=w, in_=Woc[c][:, 512:1024])
                wo1.append(w)
            for sb in range(2):
                qr = epil.tile([P, 512], f32, tag="qr")
                nc.sync.dma_start(out=qr, in_=qresc[sb][:, 512:1024])
                ps_o = ps_big.tile([P, 512], f32, tag="pb")
                for c in range(NCH):
                    nc.tensor.matmul(
                        ps_o, r(ctxt[c][:, sb * P:(sb + 1) * P]),
                        r(wo1[c]),
                        start=(c == 0), stop=(c == NCH - 1))
                nc.vector.tensor_add(out=osbs[sb][:, 512:1024], in0=ps_o,
                                     in1=qr)
                nc.vector.tensor_add(out=osbs[sb][:, 512:1024],
                                     in0=osbs[sb][:, 512:1024],
                                     in1=bob_sb[:, 512:1024])
            for sb in range(2):
                osb = osbs[sb]
                stats = epil.tile([P, 2, 6], f32, tag="stats")
                for g in range(2):
                    nc.vector.bn_stats(out=stats[:, g, :],
                                       in_=osb[:, g * 512:(g + 1) * 512])
                mv = epil.tile([P, 2], f32, tag="mv")
                nc.vector.bn_aggr(out=mv, in_=stats)
                lnl = epil.tile([P, 1], f32, tag="lnl")
                nc.scalar.activation(out=lnl, in_=mv[:, 1:2], func=AT.Ln,
                                     bias=eps_vec, scale=1.0)
                rstd = epil.tile([P, 1], f32, tag="rstd")
                nc.scalar.activation(out=rstd, in_=lnl, func=AT.Exp, scale=-0.5)
                for half in range(2):
                    hs = slice(half * 512, (half + 1) * 512)
                    nrm = epil.tile([P, 512], f32, tag="qr")
                    nc.vector.tensor_scalar(
                        out=nrm, in0=osb[:, hs], scalar1=mv[:, 0:1],
                        scalar2=rstd, op0=A.subtract, op1=A.mult)
                    fin = epil.tile([P, 512], f32, tag="qr")
                    nc.vector.tensor_mul(out=fin, in0=nrm, in1=lgb_sb[:, hs])
                    nc.vector.tensor_add(out=fin, in0=fin, in1=lbb_sb[:, hs])
                    nc.sync.dma_start(out=outc[sb][:, hs], in_=fin)

    nc.compile()
    return nc


def _prep_inputs(inputs):
    import ml_dtypes
    f = np.float32
    bf = ml_dtypes.bfloat16
    q = np.asarray(inputs["query"], f)
    k = np.asarray(inputs["key_t"], f)
    v = np.asarray(inputs["value"], f)
    host = {
        "Wq": np.ascontiguousarray(np.asarray(inputs["Wq"], f)),
        "Wk": np.ascontiguousarray(np.asarray(inputs["Wk"], f)),
        "Wv": np.ascontiguousarray(np.asarray(inputs["Wv"], f)).astype(bf),
        "Wo": np.ascontiguousarray(np.asarray(inputs["Wo"], f)),
        "Ws1": np.ascontiguousarray(np.asarray(inputs["Ws1"], f)).astype(bf),
        "Ws2": np.ascontiguousarray(np.asarray(inputs["Ws2"], f)).astype(bf),
        "bqc": np.ascontiguousarray((np.asarray(inputs["bq"], f) / np.sqrt(HD).astype(f)).reshape(NCH, P).T),
        "bkc": np.ascontiguousarray(np.asarray(inputs["bk"], f).reshape(NCH, P).T),
        "bs1r": np.asarray(inputs["bs1"], f).reshape(1, H2),
        "bs2r": np.asarray(inputs["bs2"], f).reshape(1, H),
        "bvb": np.ascontiguousarray(
            np.broadcast_to(np.asarray(inputs["bv"], f), (P, H))),
        "bob": np.ascontiguousarray(
            np.broadcast_to(np.asarray(inputs["bo"], f), (P, H))),
        "lgb": np.ascontiguousarray(
            np.broadcast_to(np.asarray(inputs["ln_g"], f), (P, H))),
        "lbb": np.ascontiguousarray(
            np.broadcast_to(np.asarray(inputs["ln_b"], f), (P, H))),
        "msbr": np.ascontiguousarray(
            np.asarray(inputs["msb"], f).reshape(P, NH * HD * HD // P)),
        "gsel": np.ascontiguousarray(
            (np.arange(P)[:, None] // 8 == np.arange(NH)[None, :]).astype(f)),
        "eye": np.eye(HD, dtype=f),
    }
    qTs = [np.ascontiguousarray(q[b].T) for b in range(B)]
    kTs = [np.ascontiguousarray(k[b].T) for b in range(B)]
    vTs = [np.ascontiguousarray(v[b].T) for b in range(B)]
    in_maps = []
    for core in range(8):
        b, j = core // QSHARD, core % QSHARD
        qs = j * QSL
        m = dict(host)
        m["qT"] = qTs[b].astype(bf)
        m["kT"] = kTs[b]
        m["vT"] = vTs[b].astype(bf)
        m["qsT"] = np.ascontiguousarray(qTs[b][:, qs:qs + QSL])
        m["qres"] = np.ascontiguousarray(q[b, qs:qs + QSL, :])
        in_maps.append(m)
    return in_maps


def kernel(**inputs):
    from concourse.bass_utils import run_bass_kernel_spmd

    if "nc" not in _CACHE:
        _CACHE["nc"] = _build()
    nc = _CACHE["nc"]
    in_maps = _prep_inputs(inputs)
    core_ids = list(range(8))
    res = run_bass_kernel_spmd(nc, in_maps, core_ids, trace=False)
    out = np.empty((B, S, H), np.float32)
    for core in range(8):
        b, j = core // QSHARD, core % QSHARD
        out[b, j * QSL:(j + 1) * QSL, :] = res.results[core]["out"]
    return out


# revision 23
# speedup vs baseline: 1.0016x; 1.0016x over previous
"""EnhancedAttention Trainium2 kernel (nn_EnhancedAttention_70068096467384).

Sharding: 8 cores = 2 batches x 4 query-slices (256 queries each).
Each core computes the full K/V projections for its batch (duplicated
within the 4-core batch group; platform collectives have ~80us fixed
overhead, more than the whole kernel target), attention for its query
slice over all 16 heads, the output projection, residual and LayerNorm,
and returns its [256, 1024] slice of the output. The host concatenates
slices -- pure data movement, no arithmetic.

Layout: activations feature-major ("transposed" [feature, token]) so
every matmul contracts over the partition dim:
  Q^T[d,q]   = Wq.T @ qslice^T         (lhsT=Wq block,   rhs=query^T slice)
  K^T[d,k]   = Wk.T @ key^T
  V[k,d]     = value^T.T @ Wv          (lhsT=value^T,    rhs=Wv block)
  s^T[k,q]   = (K^T).T @ Q^T           (per head, contraction d=64)
  ctx^T[d,q] = [V|1].T @ exp(s')       (ones column yields softmax sums)
  out[s,h]   = (ctx^T).T @ Wo          (token-major again for LayerNorm)

Gate math (per-head msb scalar a, per-batch scalar spec):
  scores' = spec * s * (1 + SP*sigmoid(a*s)),  s = Q K^T / sqrt(HD)
  with sigmoid(z) = (1+tanh(z/2))/2:
  scores' = A*s + B*s*v,  v = tanh((a/2)*s),  A = spec*(1+SP/2), B = spec*SP/2
  exp(scores') = Exp(A * g),  g = s * (1 + (B/A)*v),  B/A const = (SP/2)/(1+SP/2)
tanh and exp share one ACT table set (exp_and_others) -> no table
ping-pong. Softmax skips the row-max subtraction (scores are bounded,
|scores'| < ~3), so unnormalized exps are valid and the ones-column sums
normalize ctx. 1/sum is applied to ctx^T via a PE broadcast of the
reciprocal row. rstd for LayerNorm = Exp(-0.5*Ln(var+eps)) (ln/exp share
a table set; avoids the loose-ULP sqrt table).
"""

import numpy as np

B, S, H, NH = 2, 1024, 1024, 16
HD = H // NH            # 64
H2 = H // 2             # 512 (spec MLP hidden)
SP = 0.05
EPS = 1e-5
P = 128
NCH = H // P            # 8 feature chunks
NKB = S // P            # 8 key blocks
QSHARD = 4
QSL = S // QSHARD       # 256
BA = (SP / 2.0) / (1.0 + SP / 2.0)
AF = 1.0 + SP / 2.0
MM_DT = "float32r"      # fast fp32 matmul mode; "float32" = exact but 4x slower

_CACHE = {}


def _build(mm_dt=MM_DT):
    import concourse.bacc as bacc
    import concourse.mybir as mybir
    import concourse.tile as tile

    f32 = mybir.dt.float32
    bf16 = mybir.dt.bfloat16
    mmdt = getattr(mybir.dt, mm_dt)
    A = mybir.AluOpType
    AT = mybir.ActivationFunctionType

    def r(ap):
        return ap.bitcast(mmdt)

    nc = bacc.Bacc(None, target_bir_lowering=False, debug=False)

    def din(name, shape):
        return nc.dram_tensor(name, shape, f32, kind="ExternalInput").ap()

    def dinr(name, shape):
        return nc.dram_tensor(name, shape, mmdt, kind="ExternalInput").ap()

    def dinb(name, shape):
        return nc.dram_tensor(name, shape, bf16, kind="ExternalInput").ap()

    qT = dinb("qT", [H, S])          # query^T full (spec-MLP mean)
    qsT = dinr("qsT", [H, QSL])      # query^T slice (Q projection)
    kT = dinr("kT", [H, S])
    vT = dinb("vT", [H, S])
    qres = din("qres", [QSL, H])    # query slice token-major (residual)
    Wq, Wk, Wo = (dinr(n, [H, H]) for n in ("Wq", "Wk", "Wo"))
    Wv = dinb("Wv", [H, H])
    Ws1 = dinb("Ws1", [H, H2])
    Ws2 = dinb("Ws2", [H2, H])
    bqc = din("bqc", [P, NCH])      # bq.reshape(8,128).T
    bkc = din("bkc", [P, NCH])
    bs1r = din("bs1r", [1, H2])
    bs2r = din("bs2r", [1, H])
    bvb = din("bvb", [P, H])        # broadcasts along partitions
    bob = din("bob", [P, H])
    lgb = din("lgb", [P, H])
    lbb = din("lbb", [P, H])
    msbr = din("msbr", [P, NH * HD * HD // P])   # msb flat as [128, 512]
    gsel = din("gsel", [P, NH])     # gsel[p,h] = (p//8 == h)
    eye = din("eye", [HD, HD])
    out = nc.dram_tensor("out", [QSL, H], f32, kind="ExternalOutput").ap()

    qTc = qT.rearrange("(c p) s -> c p s", p=P)
    qsTc = qsT.rearrange("(c p) s -> c p s", p=P)
    kTc = kT.rearrange("(c p) s -> c p s", p=P)
    vTc = vT.rearrange("(c p) s -> c p s", p=P)
    Wqc = Wq.rearrange("(c p) n -> c p n", p=P)
    Wkc = Wk.rearrange("(c p) n -> c p n", p=P)
    Wvc = Wv.rearrange("(c p) n -> c p n", p=P)
    Woc = Wo.rearrange("(c p) n -> c p n", p=P)
    Ws1c = Ws1.rearrange("(c p) n -> c p n", p=P)
    Ws2c = Ws2.rearrange("(c p) n -> c p n", p=P)
    qresc = qres.rearrange("(c p) n -> c p n", p=P)
    outc = out.rearrange("(c p) n -> c p n", p=P)

    from contextlib import ExitStack

    with tile.TileContext(nc) as tc:
        with ExitStack() as ctx:
            ec = ctx.enter_context
            consts = ec(tc.tile_pool(name="consts", bufs=1))
            actin = ec(tc.tile_pool(name="actin", bufs=8))
            qsin = ec(tc.tile_pool(name="qsin", bufs=NCH))
            wstr = ec(tc.tile_pool(name="wstr", bufs=8))
            ktp = ec(tc.tile_pool(name="ktp", bufs=NCH))
            vaugp = ec(tc.tile_pool(name="vaugp", bufs=NKB))
            qtp = ec(tc.tile_pool(name="qtp", bufs=NCH))
            ctxp = ec(tc.tile_pool(name="ctxp", bufs=NCH))
            gate3 = ec(tc.tile_pool(name="gate3", bufs=3))
            gate2 = ec(tc.tile_pool(name="gate2", bufs=1))
            pexp = ec(tc.tile_pool(name="pexp", bufs=3))
            smalls = ec(tc.tile_pool(name="smalls", bufs=1))
            epil = ec(tc.tile_pool(name="epil", bufs=2))
            wmlp = ec(tc.tile_pool(name="wmlp", bufs=2))
            ps_sc = ec(tc.tile_pool(name="ps_sc", bufs=3, space="PSUM"))
            ps_pv = ec(tc.tile_pool(name="ps_pv", bufs=2, space="PSUM"))
            ps_big = ec(tc.tile_pool(name="ps_big", bufs=2, space="PSUM"))
            ps_sm = ec(tc.tile_pool(name="ps_sm", bufs=1, space="PSUM"))
            ps_bc = ps_sm
            # ---------------- constants ----------------
            ones64 = consts.tile([P, HD], f32)
            nc.vector.memset(ones64, 1.0)
            onesrow = consts.tile([1, P], f32)
            nc.vector.memset(onesrow, 1.0)
            one1 = consts.tile([1, 1], f32)
            nc.vector.memset(one1, 1.0)
            eps_vec = consts.tile([P, 1], f32)
            nc.vector.memset(eps_vec, EPS)
            bq_sb = consts.tile([P, NCH], f32)
            nc.sync.dma_start(out=bq_sb, in_=bqc)
            bk_sb = consts.tile([P, NCH], f32)
            nc.sync.dma_start(out=bk_sb, in_=bkc)
            bs1_sb = consts.tile([1, H2], f32)
            nc.sync.dma_start(out=bs1_sb, in_=bs1r)
            bs2_sb = consts.tile([1, H], f32)
            nc.sync.dma_start(out=bs2_sb, in_=bs2r)
            gsel_sb = consts.tile([P, NH], f32)
            nc.sync.dma_start(out=gsel_sb, in_=gsel)
            eye_sb = consts.tile([HD, HD], f32)
            nc.sync.dma_start(out=eye_sb, in_=eye)
            onescol = consts.tile([P, NH, 1], bf16)
            nc.vector.memset(onescol, 1.0)

            # -------- msb head scalars: ah[:,h] = mean(msb[h]) / 2 --------
            msb_sb = smalls.tile([P, NH * HD * HD // P], f32, tag="sm")
            nc.sync.dma_start(out=msb_sb, in_=msbr)
            mpart = smalls.tile([P, 1], f32, tag="sm2")
            nc.vector.tensor_reduce(out=mpart, in_=msb_sb, op=A.add,
                                    axis=mybir.AxisListType.X)
            # scale by 1/(HD*HD) * 1/2 now (per-partition partial sums)
            nc.vector.tensor_scalar_mul(mpart, mpart, 0.5 / (HD * HD))
            ps_mh = ps_sm.tile([P, 512], f32, tag="ps")
            nc.tensor.matmul(ps_mh[0:NH, 0:1], gsel_sb, mpart,
                             start=True, stop=True)
            mh16 = smalls.tile([16, 1], f32, tag="sm3")
            nc.vector.tensor_copy(out=mh16, in_=ps_mh[0:NH, 0:1])
            ps_mr = ps_sm.tile([P, 512], f32, tag="ps")
            nc.tensor.matmul(ps_mr[0:1, 0:NH], mh16, eye_sb[0:NH, 0:NH],
                             start=True, stop=True)
            mrow = smalls.tile([1, NH], f32, tag="sm4")
            nc.vector.tensor_copy(out=mrow, in_=ps_mr[0:1, 0:NH])
            ps_ah = ps_sm.tile([P, 512], f32, tag="ps")
            nc.tensor.matmul(ps_ah[:, 0:NH], onesrow, mrow, start=True, stop=True)
            ah_sb = consts.tile([P, NH], f32)
            nc.vector.tensor_copy(out=ah_sb, in_=ps_ah[:, 0:NH])

            # -------- query^T slice (Q proj input) and full (spec mean) --------
            qs_in = []
            for c in range(NCH):
                t = qsin.tile([P, QSL], mmdt, tag="qs")
                nc.sync.dma_start(out=t, in_=qsTc[c])
                qs_in.append(t)
            sin_col = smalls.tile([P, NCH], bf16, tag="sin")
            with nc.allow_low_precision(
                    reason="spec-MLP input mean; feeds a sigmoid-mean scalar"):
                for c in range(NCH):
                    t = actin.tile([P, S], bf16, tag="act")
                    nc.sync.dma_start(out=t, in_=qTc[c])
                    nc.vector.tensor_reduce(out=sin_col[:, c:c + 1], in_=t,
                                            op=A.add, axis=mybir.AxisListType.X)

            # -------- Q^T projection (+bias, x 1/sqrt(HD)) --------
            qt = [qtp.tile([P, QSL], mmdt, tag="qt", name=f"qt{i}") for i in range(NCH)]
            for dh in range(2):
                wblk = []
                for c in range(NCH):
                    w = wstr.tile([P, 512], mmdt, tag="w")
                    nc.sync.dma_start(out=w, in_=Wqc[c][:, dh * 512:(dh + 1) * 512])
                    wblk.append(w)
                for dbl in range(4):
                    db = dh * 4 + dbl
                    ps_q = ps_big.tile([P, 512], f32, tag="pb")
                    for c in range(NCH):
                        nc.tensor.matmul(
                            ps_q[:, 0:QSL],
                            r(wblk[c][:, dbl * P:(dbl + 1) * P]),
                            r(qs_in[c]),
                            start=(c == 0), stop=(c == NCH - 1))
                    nc.scalar.activation(
                        out=qt[db], in_=ps_q[:, 0:QSL], func=AT.Identity,
                        bias=bq_sb[:, db:db + 1], scale=1.0 / np.sqrt(HD))

            # -------- spec MLP --------
            ps_m1 = ps_big.tile([P, 512], f32, tag="pb")
            for c in range(NCH):
                w = wmlp.tile([P, 512], bf16, tag="wm")
                nc.sync.dma_start(out=w, in_=Ws1c[c])
                nc.tensor.matmul(ps_m1[0:1, :], sin_col[:, c:c + 1], w,
                                 start=(c == 0), stop=(c == NCH - 1))
            h1row = smalls.tile([1, H2], f32, tag="h1r")
            nc.vector.scalar_tensor_tensor(
                out=h1row, in0=ps_m1[0:1, :], scalar=1.0 / S, in1=bs1_sb,
                op0=A.mult, op1=A.add)
            h1c = smalls.tile([P, 4], bf16, tag="h1c")
            for c in range(4):
                ps_tr = ps_sm.tile([P, 512], f32, tag="ps")
                nc.tensor.matmul(ps_tr[:, 0:1],
                                 h1row[0:1, c * P:(c + 1) * P], one1,
                                 start=True, stop=True)
                nc.vector.tensor_copy(out=h1c[:, c:c + 1], in_=ps_tr[:, 0:1])
            nc.vector.tensor_scalar_max(h1c, h1c, 0.0)
            zrow = smalls.tile([1, H], f32, tag="zr")
            for half in range(2):
                ps_m2 = ps_big.tile([P, 512], f32, tag="pb")
                for c in range(4):
                    w = wmlp.tile([P, 512], bf16, tag="wm")
                    nc.sync.dma_start(out=w, in_=Ws2c[c][:, half * 512:(half + 1) * 512])
                    nc.tensor.matmul(ps_m2[0:1, :], h1c[:, c:c + 1], w,
                                     start=(c == 0), stop=(c == 3))
                nc.vector.tensor_add(
                    out=zrow[0:1, half * 512:(half + 1) * 512],
                    in0=ps_m2[0:1, :],
                    in1=bs2_sb[0:1, half * 512:(half + 1) * 512])
            zsig = smalls.tile([1, H], f32, tag="sm")
            nc.scalar.activation(out=zsig, in_=zrow, func=AT.Sigmoid)
            zsum = smalls.tile([1, 1], f32, tag="zsum")
            nc.vector.tensor_reduce(out=zsum, in_=zsig, op=A.add,
                                    axis=mybir.AxisListType.X)
            ps_sp = ps_sm.tile([P, 512], f32, tag="ps")
            nc.tensor.matmul(ps_sp[:, 0:1], onesrow, zsum, start=True, stop=True)
            a_vec = consts.tile([P, 1], f32)
            nc.vector.tensor_scalar_mul(a_vec, ps_sp[:, 0:1], AF / H)

            # -------- V projection -> V_aug = per head [V|1] / [1|V] --------
            bvb_sb = consts.tile([P, H], f32)
            nc.sync.dma_start(out=bvb_sb, in_=bvb)
            vt_in = []
            for c in range(NCH):
                t = actin.tile([P, S], bf16, tag="act")
                nc.sync.dma_start(out=t, in_=vTc[c])
                vt_in.append(t)
            vaug = [vaugp.tile([P, NH, HD + 1], bf16, tag="va", name=f"va{i}") for i in range(NKB)]
            for kb in range(NKB):
                nc.vector.tensor_copy(out=vaug[kb][:, :, HD:HD + 1],
                                      in_=onescol)                 # [V_h | 1]
            for dh in range(2):
                wblk = []
                for c in range(NCH):
                    w = wstr.tile([P, 512], bf16, tag="w")
                    nc.sync.dma_start(out=w, in_=Wvc[c][:, dh * 512:(dh + 1) * 512])
                    wblk.append(w)
                for kb in range(NKB):
                    ps_v = ps_big.tile([P, 512], f32, tag="pb")
                    for c in range(NCH):
                        nc.tensor.matmul(
                            ps_v,
                            vt_in[c][:, kb * P:(kb + 1) * P],
                            wblk[c],
                            start=(c == 0), stop=(c == NCH - 1))
                    psv = ps_v.rearrange("p (g w) -> p g w", w=HD)
                    bvv = bvb_sb[:, dh * 512:(dh + 1) * 512].rearrange(
                        "p (g w) -> p g w", w=HD)
                    nc.vector.tensor_add(
                        out=vaug[kb][:, dh * 8:dh * 8 + 8, 0:HD],
                        in0=psv, in1=bvv)

            # -------- K^T projection (+bias) --------
            kt_in = []
            for c in range(NCH):
                t = actin.tile([P, S], mmdt, tag="act")
                nc.sync.dma_start(out=t, in_=kTc[c])
                kt_in.append(t)
            kt = [ktp.tile([P, S], mmdt, tag="kt", name=f"kt{i}") for i in range(NCH)]
            for dh in range(2):
                wblk = []
                for c in range(NCH):
                    w = wstr.tile([P, 512], mmdt, tag="w")
                    nc.sync.dma_start(out=w, in_=Wkc[c][:, dh * 512:(dh + 1) * 512])
                    wblk.append(w)
                for dbl in range(4):
                    db = dh * 4 + dbl
                    for kh in range(2):
                        ps_k = ps_big.tile([P, 512], f32, tag="pb")
                        for c in range(NCH):
                            nc.tensor.matmul(
                                ps_k,
                                r(wblk[c][:, dbl * P:(dbl + 1) * P]),
                                r(kt_in[c][:, kh * 512:(kh + 1) * 512]),
                                start=(c == 0), stop=(c == NCH - 1))
                        nc.scalar.activation(
                            out=kt[db][:, kh * 512:(kh + 1) * 512], in_=ps_k,
                            func=AT.Identity, bias=bk_sb[:, db:db + 1],
                            scale=1.0)

            # -------- attention heads (+ interleaved out-proj half 0) --------
            wo0 = []
            for c in range(NCH):
                w = wstr.tile([P, 512], mmdt, tag="w")
                nc.sync.dma_start(out=w, in_=Woc[c][:, 0:512])
                wo0.append(w)
            pso0 = [ps_big.tile([P, 512], f32, tag="pb", name=f"pso0_{i}")
                    for i in range(2)]
            ctxt = [ctxp.tile([P, QSL], mmdt, tag="ctx", name=f"ctx{i}") for i in range(NCH)]
            for h in range(NH):
                ch, off = h // 2, (h % 2) * HD
                even = (h % 2 == 0)
                pv_ps = ps_pv.tile([P, QSL], f32, tag="pv")
                for kp in range(NKB // 2):
                    s_ps = ps_sc.tile([P, 2 * QSL], f32, tag="sc")
                    for j in range(2):
                        kb = 2 * kp + j
                        nc.tensor.matmul(
                            s_ps[:, j * QSL:(j + 1) * QSL],
                            r(kt[ch][off:off + HD, kb * P:(kb + 1) * P]),
                            r(qt[ch][off:off + HD, :]), start=True, stop=True)
                    v_sb = gate3.tile([P, 2 * QSL], bf16, tag="v")
                    nc.scalar.activation(out=v_sb, in_=s_ps, func=AT.Tanh,
                                         scale=ah_sb[:, h:h + 1])
                    w1_sb = gate3.tile([P, 2 * QSL], bf16, tag="w1")
                    nc.vector.tensor_scalar(
                        out=w1_sb, in0=v_sb, scalar1=BA, scalar2=1.0,
                        op0=A.mult, op1=A.add)
                    g_sb = gate3.tile([P, 2 * QSL], f32, tag="g")
                    nc.vector.tensor_mul(out=g_sb, in0=s_ps, in1=w1_sb)
                    p_sb = pexp.tile([P, 2 * QSL], bf16, tag="p")
                    nc.scalar.activation(out=p_sb, in_=g_sb, func=AT.Exp,
                                         scale=a_vec)
                    for j in range(2):
                        kb = 2 * kp + j
                        lh = vaug[kb].rearrange("p h w -> p (h w)")
                        nc.tensor.matmul(
                            pv_ps[0:HD + 1, :],
                            lh[:, h * (HD + 1):(h + 1) * (HD + 1)],
                            p_sb[:, j * QSL:(j + 1) * QSL],
                            start=(kb == 0), stop=(kb == NKB - 1))
                # normalize ctx rows by softmax sums (row HD of pv_ps)
                inv_sb = gate2.tile([P, QSL], f32, tag="inv")
                nc.vector.reciprocal(out=inv_sb[HD:HD + 1, :],
                                     in_=pv_ps[HD:HD + 1, :])
                bc_ps = ps_bc.tile([P, 2 * QSL], f32, tag="ps")
                nc.tensor.matmul(
                    bc_ps[0:HD, 0:QSL], ones64[HD:HD + 1, 0:HD],
                    inv_sb[HD:HD + 1, :], start=True, stop=True)
                bc_sb = gate2.tile([P, QSL], f32, tag="bcs")
                nc.vector.tensor_copy(out=bc_sb[0:HD, :], in_=bc_ps[0:HD, 0:QSL])
                if even:
                    nc.vector.tensor_mul(
                        out=ctxt[ch][0:HD, :],
                        in0=pv_ps[0:HD, :], in1=bc_sb[0:HD, :])
                else:
                    # scale into a temp, then PE-shift to partitions 64..127
                    cso = gate2.tile([P, QSL], f32, tag="cso")
                    nc.vector.tensor_mul(
                        out=cso[0:HD, :], in0=pv_ps[0:HD, :],
                        in1=bc_sb[0:HD, :])
                    sh_ps = ps_bc.tile([P, 2 * QSL], f32, tag="ps")
                    nc.tensor.matmul(
                        sh_ps[HD:P, 0:QSL], eye_sb, cso[0:HD, :],
                        start=True, stop=True)
                    nc.vector.tensor_copy(out=ctxt[ch][HD:P, :],
                                          in_=sh_ps[HD:P, 0:QSL])
                if not even:
                    for sb in range(2):
                        nc.tensor.matmul(
                            pso0[sb], r(ctxt[ch][:, sb * P:(sb + 1) * P]),
                            r(wo0[ch]),
                            start=(ch == 0), stop=(ch == NCH - 1))

            # -------- output projection + residual + LayerNorm --------
            bob_sb = consts.tile([P, H], f32)
            nc.sync.dma_start(out=bob_sb, in_=bob)
            lgb_sb = consts.tile([P, H], f32)
            nc.sync.dma_start(out=lgb_sb, in_=lgb)
            lbb_sb = consts.tile([P, H], f32)
            nc.sync.dma_start(out=lbb_sb, in_=lbb)
            osbs = []
            for sb in range(2):
                osbs.append(epil.tile([P, H], f32, tag="osb", name=f"osb{sb}"))
            for sb in range(2):
                qr = epil.tile([P, 512], f32, tag="qr")
                nc.sync.dma_start(out=qr, in_=qresc[sb][:, 0:512])
                nc.vector.tensor_add(out=osbs[sb][:, 0:512], in0=pso0[sb],
                                     in1=qr)
                nc.vector.tensor_add(out=osbs[sb][:, 0:512],
                                     in0=osbs[sb][:, 0:512],
                                     in1=bob_sb[:, 0:512])
            wo1 = []
            for c in range(NCH):
                w = wstr.tile([P, 512], mmdt, tag="w")
                nc.sync.dma_start(out=w, in_=Woc[c][:, 512:1024])
                wo1.append(w)
            for sb in range(2):
                qr = epil.tile([P, 512], f32, tag="qr")
                nc.sync.dma_start(out=qr, in_=qresc[sb][:, 512:1024])
                ps_o = ps_big.tile([P, 512], f32, tag="pb")
                for c in range(NCH):
                    nc.tensor.matmul(
                        ps_o, r(ctxt[c][:, sb * P:(sb + 1) * P]),
                        r(wo1[c]),
                        start=(c == 0), stop=(c == NCH - 1))
                nc.vector.tensor_add(out=osbs[sb][:, 512:1024], in0=ps_o,
                                     in1=qr)
                nc.vector.tensor_add(out=osbs[sb][:, 512:1024],
                                     in0=osbs[sb][:, 512:1024],
                                     in1=bob_sb[:, 512:1024])
            for sb in range(2):
                osb = osbs[sb]
                stats = epil.tile([P, 2, 6], f32, tag="stats")
                for g in range(2):
                    nc.vector.bn_stats(out=stats[:, g, :],
                                       in_=osb[:, g * 512:(g + 1) * 512])
                mv = epil.tile([P, 2], f32, tag="mv")
                nc.vector.bn_aggr(out=mv, in_=stats)
                lnl = epil.tile([P, 1], f32, tag="lnl")
                nc.scalar.activation(out=lnl, in_=mv[:, 1:2], func=AT.Ln,
                                     bias=eps_vec, scale=1.0)
                rstd = epil.tile([P, 1], f32, tag="rstd")
                nc.scalar.activation(out=rstd, in_=lnl, func=AT.Exp, scale=-0.5)
                for half in range(2):
                    hs = slice(half * 512, (half + 1) * 512)
                    nrm = epil.tile([P, 512], f32, tag="qr")
                    nc.vector.tensor_scalar(
                        out=nrm, in0=osb[:, hs], scalar1=mv[:, 0:1],
                        scalar2=rstd, op0=A.subtract, op1=A.mult)
                    fin = epil.tile([P, 512], f32, tag="qr")
                    nc.vector.tensor_mul(out=fin, in0=nrm, in1=lgb_sb[:, hs])
                    nc.vector.tensor_add(out=fin, in0=fin, in1=lbb_sb[:, hs])
                    nc.sync.dma_start(out=outc[sb][:, hs], in_=fin)

    nc.compile()
    return nc


def _prep_inputs(inputs):
    import ml_dtypes
    f = np.float32
    bf = ml_dtypes.bfloat16
    q = np.asarray(inputs["query"], f)
    k = np.asarray(inputs["key_t"], f)
    v = np.asarray(inputs["value"], f)
    host = {
        "Wq": np.ascontiguousarray(np.asarray(inputs["Wq"], f)),
        "Wk": np.ascontiguousarray(np.asarray(inputs["Wk"], f)),
        "Wv": np.ascontiguousarray(np.asarray(inputs["Wv"], f)).astype(bf),
        "Wo": np.ascontiguousarray(np.asarray(inputs["Wo"], f)),
        "Ws1": np.ascontiguousarray(np.asarray(inputs["Ws1"], f)).astype(bf),
        "Ws2": np.ascontiguousarray(np.asarray(inputs["Ws2"], f)).astype(bf),
        "bqc": np.ascontiguousarray((np.asarray(inputs["bq"], f) / np.sqrt(HD).astype(f)).reshape(NCH, P).T),
        "bkc": np.ascontiguousarray(np.asarray(inputs["bk"], f).reshape(NCH, P).T),
        "bs1r": np.asarray(inputs["bs1"], f).reshape(1, H2),
        "bs2r": np.asarray(inputs["bs2"], f).reshape(1, H),
        "bvb": np.ascontiguousarray(
            np.broadcast_to(np.asarray(inputs["bv"], f), (P, H))),
        "bob": np.ascontiguousarray(
            np.broadcast_to(np.asarray(inputs["bo"], f), (P, H))),
        "lgb": np.ascontiguousarray(
            np.broadcast_to(np.asarray(inputs["ln_g"], f), (P, H))),
        "lbb": np.ascontiguousarray(
            np.broadcast_to(np.asarray(inputs["ln_b"], f), (P, H))),
        "msbr": np.ascontiguousarray(
            np.asarray(inputs["msb"], f).reshape(P, NH * HD * HD // P)),
        "gsel": np.ascontiguousarray(
            (np.arange(P)[:, None] // 8 == np.arange(NH)[None, :]).astype(f)),
        "eye": np.eye(HD, dtype=f),
    }
    qTs = [np.ascontiguousarray(q[b].T) for b in range(B)]
    kTs = [np.ascontiguousarray(k[b].T) for b in range(B)]
    vTs = [np.ascontiguousarray(v[b].T) for b in range(B)]
    in_maps = []
    for core in range(8):
        b, j = core // QSHARD, core % QSHARD
        qs = j * QSL
        m = dict(host)
        m["qT"] = qTs[b].astype(bf)
        m["kT"] = kTs[b]
        m["vT"] = vTs[b].astype(bf)
        m["qsT"] = np.ascontiguousarray(qTs[b][:, qs:qs + QSL])
        m["qres"] = np.ascontiguousarray(q[b, qs:qs + QSL, :])
        in_maps.append(m)
    return in_maps


def kernel(**inputs):
    from concourse.bass_utils import run_bass_kernel_spmd

    if "nc" not in _CACHE:
        _CACHE["nc"] = _build()
    nc = _CACHE["nc"]
    in_maps = _prep_inputs(inputs)
    core_ids = list(range(8))
    res = run_bass_kernel_spmd(nc, in_maps, core_ids, trace=False)
    out = np.empty((B, S, H), np.float32)
    for core in range(8):
        b, j = core // QSHARD, core % QSHARD
        out[b, j * QSL:(j + 1) * QSL, :] = res.results[core]["out"]
    return out
